# revision 9
# baseline (speedup 1.0000x reference)
"""DIoU loss (nms_detection) Trainium2 kernel.

Reference semantics: pairwise IoU [P,T] between pred_boxes (xyxy interp) and
target_boxes, argmax over targets per pred row (first-occurrence tie-break),
gather matched targets, DIoU (cxcywh interp) per row, loss = 1 - mean(diou).

Key algorithmic facts exploited (validated against the jax reference):
 1. A box with w<=0 or h<=0 (75% of uniform-random "boxes") has zero
    intersection with everything -> its whole IoU row/column is +-0.  A row
    whose max IoU is <= 0 argmaxes to index 0 (first occurrence among ties).
    So only non-degenerate preds x non-degenerate targets need the dense
    pairwise block (~1/16 of the matrix), and every row with max<=0 matches
    target_boxes[0].
 2. iou = inter/(pa+ta-inter+eps) is a monotone function of
    iou' = inter/(pa+ta+eps) (g = f/(1+f)), so argmax(iou) == argmax(iou').
    This removes one full-width pass (the union) from the hot loop.

Sharding: compacted non-degenerate pred rows are split evenly across the 8
cores (data-parallel rows of the pairwise block, per the sharding hint); each
core holds the full compacted target set, computes its block + local DIoU
partial sums; the final (tiny) reduction over 8x[128,16] partials is done on
host, equivalent to the suggested all-reduce of the mean.
"""

import os
import numpy as np
from contextlib import ExitStack

import concourse.bass as bass
import concourse.bacc as bacc
import concourse.mybir as mybir
from concourse import tile
from concourse.bass_utils import run_bass_kernel_spmd

N_CORES = 8
T_CAP = 1280      # compacted-target capacity (multiple of 128)
P_TILES = 3       # compacted-pred tiles per core -> 384 rows/core, 3072 total
P_CAP_CORE = 128 * P_TILES
EPS = np.float32(1e-7)
F32 = mybir.dt.float32
U32 = mybir.dt.uint32

# ---------------------------------------------------------------------------
# Custom DVE ops (registered at import; names unique, appended after stock OPS)
# ---------------------------------------------------------------------------
from concourse.dve_spec import Spec, Src0, Src1, C0, C1, lower, relu, minn, maxx, AluOp
import concourse.dve_ops as dve_ops
from concourse.dve_ops import DveOp, OPS
from concourse.dve_uop import DveOpSpec


def _register_dve(name, spec):
    for op in OPS:
        if op.name == name:
            return op
    shas = {}
    for ver in ("v3", "v4"):
        uops = lower(spec, ver=ver)
        shas[ver] = DveOpSpec(name=name, opcode=0, uops=uops, rd1_en=True).sha(ver)
    op = DveOp(name, spec, subdim=False, uops_sha=shas)
    OPS.append(op)
    dve_ops.CUSTOM_DVE_SPECS[name] = spec
    dve_ops._SUB_OPCODE_FOR_NAME[name] = dve_ops._CUSTOM_DVE_ROW_BASE + len(OPS) - 1
    assert dve_ops._SUB_OPCODE_FOR_NAME[name] < 0x20
    return op


# relu(min(Src0, s0) - max(Src1, s1)): clipped 1-D interval overlap in one pass
IXREL = _register_dve(
    "IXREL_ANT",
    Spec(
        body=relu(minn(Src0, C0) - maxx(Src1, C1)),
        reference=lambda in0, in1, s0, s1, imm2: np.maximum(
            np.minimum(in0, s0) - np.maximum(in1, s1), 0.0
        ),
    ),
)

# out = Src0*Src1, accum_out = max(out) (tensor_tensor_reduce crashes the
# device on this toolchain; this custom op is the working replacement)
MUL_RMAX = _register_dve(
    "MUL_RMAX_ANT",
    Spec(
        body=Src0 * Src1,
        accum=AluOp.MAX,
        reference=lambda in0, in1, s0, s1, imm2: in0 * in1,
    ),
)

_BUILD_CACHE = {}


def _build_program():
    key = (T_CAP, P_TILES)
    if key in _BUILD_CACHE:
        return _BUILD_CACHE[key]

    nc = bacc.Bacc("TRN2", target_bir_lowering=False, debug=False,
                   num_devices=N_CORES)

    din = lambda n, s: nc.dram_tensor(n, s, F32, kind="ExternalInput").ap()
    tx1r_d = din("tx1r", [128, T_CAP])
    ty1r_d = din("ty1r", [128, T_CAP])
    tx2r_d = din("tx2r", [128, T_CAP])
    ty2r_d = din("ty2r", [128, T_CAP])
    tar_d = din("tar", [128, T_CAP])
    predc_d = din("predc", [128, 8 * P_TILES])   # [x1,y1,x2,y2,pa_eps,valid,0,0]*3
    predf_d = din("predf", [128, 64])            # 8 tiles x [cx,cy,w,h,notnd,0,0,0]
    ctab_d = din("ctab", [T_CAP + 1, 4])         # compacted targets + tgt0 at row T_CAP
    tgt0_d = din("tgt0", [128, 12])  # [cx,cy,w,h,ltx,lty,rbx,rby,ta0,0,0,0]
    out_d = nc.dram_tensor("acc", [128, P_TILES + 8], F32, kind="ExternalOutput").ap()

    with tile.TileContext(nc) as tc, ExitStack() as ctx:
        rep = ctx.enter_context(tc.tile_pool(name="rep", bufs=1))
        big = ctx.enter_context(tc.tile_pool(name="big", bufs=2))
        sml = ctx.enter_context(tc.tile_pool(name="sml", bufs=1))

        tx1r = rep.tile([128, T_CAP], F32, tag="tx1r", name="tx1r")
        ty1r = rep.tile([128, T_CAP], F32, tag="ty1r", name="ty1r")
        tx2r = rep.tile([128, T_CAP], F32, tag="tx2r", name="tx2r")
        ty2r = rep.tile([128, T_CAP], F32, tag="ty2r", name="ty2r")
        tar = rep.tile([128, T_CAP], F32, tag="tar", name="tar")
        predc = sml.tile([128, 8 * P_TILES], F32, tag="predc", name="predc")
        predf = sml.tile([128, 64], F32, tag="predf", name="predf")
        tgt0 = sml.tile([128, 12], F32, tag="tgt0", name="tgt0")
        nc.sync.dma_start(out=tx1r[:], in_=tx1r_d[:])
        nc.sync.dma_start(out=ty1r[:], in_=ty1r_d[:])
        nc.sync.dma_start(out=tx2r[:], in_=tx2r_d[:])
        nc.sync.dma_start(out=ty2r[:], in_=ty2r_d[:])
        nc.sync.dma_start(out=tar[:], in_=tar_d[:])
        nc.sync.dma_start(out=predc[:], in_=predc_d[:])
        nc.sync.dma_start(out=predf[:], in_=predf_d[:])
        nc.sync.dma_start(out=tgt0[:], in_=tgt0_d[:])

        selall = sml.tile([128, P_TILES], F32, tag="selall", name="selall")
        gtall = sml.tile([128, 4 * P_TILES], F32, tag="gtall", name="gtall")
        outt = sml.tile([128, P_TILES + 8], F32, tag="outt", name="outt")

        TS = mybir.AluOpType

        # ---- pairwise block: per 128-row pred tile ----
        for i in range(P_TILES):
            px1 = predc[:, 8 * i + 0 : 8 * i + 1]
            py1 = predc[:, 8 * i + 1 : 8 * i + 2]
            px2 = predc[:, 8 * i + 2 : 8 * i + 3]
            py2 = predc[:, 8 * i + 3 : 8 * i + 4]
            pae = predc[:, 8 * i + 4 : 8 * i + 5]

            S = big.tile([128, T_CAP], F32, tag="S", name="S")
            J0 = big.tile([128, T_CAP], F32, tag="J0", name="J0")
            J = big.tile([128, T_CAP], F32, tag="J", name="J")
            ix = big.tile([128, T_CAP], F32, tag="ix", name="ix")
            iy = big.tile([128, T_CAP], F32, tag="iy", name="iy")
            inter = big.tile([128, T_CAP], F32, tag="inter", name="inter")
            iou = big.tile([128, T_CAP], F32, tag="iou", name="iou")
            m = sml.tile([128, 1], F32, tag=f"m{i}", name=f"m{i}")
            m8 = sml.tile([128, 8], F32, tag=f"m8{i}", name=f"m8{i}")
            ti8 = sml.tile([128, 8], U32, tag=f"ti8{i}", name=f"ti8{i}")
            idxf = sml.tile([128, 1], F32, tag=f"idxf{i}", name=f"idxf{i}")
            idxs = sml.tile([128, 1], F32, tag=f"idxs{i}", name=f"idxs{i}")
            idxu = sml.tile([128, 1], U32, tag=f"idxu{i}", name=f"idxu{i}")

            # S = max(tar + (pa+eps), eps)  (>0 wherever it matters)
            nc.vector.tensor_scalar(out=S[:], in0=tar[:], scalar1=pae,
                                    scalar2=float(EPS), op0=TS.add, op1=TS.max)
            nc.vector.reciprocal_approx_fast(out=J0[:], in_=S[:])
            nc.vector._custom_dve(  # one Newton step -> ~2 ULP
                dve_ops.RECIPROCAL_APPROX_NR, out=J[:], in0=S[:], in1=J0[:], s0=2.0
            )
            # ix = relu(min(px2,tx2) - max(px1,tx1)); same for y
            nc.vector._custom_dve(IXREL, out=ix[:], in0=tx2r[:], in1=tx1r[:],
                                  s0=px2, s1=px1)
            nc.vector._custom_dve(IXREL, out=iy[:], in0=ty2r[:], in1=ty1r[:],
                                  s0=py2, s1=py1)
            nc.vector.tensor_tensor(out=inter[:], in0=ix[:], in1=iy[:], op=TS.mult)
            # iou' = inter * (1/S); m = row max
            nc.vector._custom_dve(MUL_RMAX, out=iou[:], in0=inter[:], in1=J[:],
                                  accum_out=m[:])
            # first-occurrence argmax of iou' (== ref argmax when m > 0)
            nc.vector.tensor_copy(out=m8[:], in_=m[:, 0:1].broadcast_to((128, 8)))
            nc.vector.max_index(out=ti8[:], in_max=m8[:], in_values=iou[:])
            # sel = (m > 0); idx = sel ? argmax : T_CAP (row T_CAP of ctab = tgt0)
            nc.vector.tensor_scalar(out=selall[:, i : i + 1], in0=m[:], scalar1=0.0,
                                    scalar2=None, op0=TS.is_gt)
            nc.vector.tensor_copy(out=idxf[:], in_=ti8[:, 0:1])
            nc.vector.tensor_scalar(out=idxs[:], in0=idxf[:], scalar1=float(T_CAP),
                                    scalar2=None, op0=TS.subtract)
            nc.vector.tensor_tensor(out=idxs[:], in0=idxs[:],
                                    in1=selall[:, i : i + 1], op=TS.mult)
            nc.vector.tensor_scalar(out=idxs[:], in0=idxs[:], scalar1=float(T_CAP),
                                    scalar2=None, op0=TS.add)
            nc.vector.tensor_copy(out=idxu[:], in_=idxs[:])
            nc.gpsimd.indirect_dma_start(
                out=gtall[:, 4 * i : 4 * i + 4], out_offset=None, in_=ctab_d[:],
                in_offset=bass.IndirectOffsetOnAxis(ap=idxu[:, 0:1], axis=0),
            )

        # ---- DIoU tail A: compacted rows vs gathered targets, [128, P_TILES] ----
        NT = P_TILES
        pcx = predc[:, 0 : 8 * NT : 8]
        pcy = predc[:, 1 : 8 * NT : 8]
        pw = predc[:, 2 : 8 * NT : 8]
        ph = predc[:, 3 : 8 * NT : 8]
        vld = predc[:, 5 : 8 * NT : 8]
        tcx = gtall[:, 0 : 4 * NT : 4]
        tcy = gtall[:, 1 : 4 * NT : 4]
        tw = gtall[:, 2 : 4 * NT : 4]
        th = gtall[:, 3 : 4 * NT : 4]

        def t3(tag):
            return sml.tile([128, NT], F32, tag=tag, name=tag)

        phw, phh, thw, thh = t3("phw"), t3("phh"), t3("thw"), t3("thh")
        nc.vector.tensor_scalar(out=phw[:], in0=pw, scalar1=0.5, scalar2=None, op0=TS.mult)
        nc.vector.tensor_scalar(out=phh[:], in0=ph, scalar1=0.5, scalar2=None, op0=TS.mult)
        nc.vector.tensor_scalar(out=thw[:], in0=tw, scalar1=0.5, scalar2=None, op0=TS.mult)
        nc.vector.tensor_scalar(out=thh[:], in0=th, scalar1=0.5, scalar2=None, op0=TS.mult)
        pltx, plty, prbx, prby = t3("pltx"), t3("plty"), t3("prbx"), t3("prby")
        tltx, tlty, trbx, trby = t3("tltx"), t3("tlty"), t3("trbx"), t3("trby")
        nc.vector.tensor_tensor(out=pltx[:], in0=pcx, in1=phw[:], op=TS.subtract)
        nc.vector.tensor_tensor(out=prbx[:], in0=pcx, in1=phw[:], op=TS.add)
        nc.vector.tensor_tensor(out=plty[:], in0=pcy, in1=phh[:], op=TS.subtract)
        nc.vector.tensor_tensor(out=prby[:], in0=pcy, in1=phh[:], op=TS.add)
        nc.vector.tensor_tensor(out=tltx[:], in0=tcx, in1=thw[:], op=TS.subtract)
        nc.vector.tensor_tensor(out=trbx[:], in0=tcx, in1=thw[:], op=TS.add)
        nc.vector.tensor_tensor(out=tlty[:], in0=tcy, in1=thh[:], op=TS.subtract)
        nc.vector.tensor_tensor(out=trby[:], in0=tcy, in1=thh[:], op=TS.add)

        def diou_common(pltx, plty, prbx, prby, tltx, tlty, trbx, trby,
                        pcx, pcy, tcx, tcy, pa, ta, n, mk, ts_tgt):
            """Emits diou into a [128, n] tile and returns it.

            All args are ready-to-use APs.  ts_tgt: True when the t* box
            sides / centers / area are per-partition [128,1] scalars
            (tensor_scalar path), False when they are [128, n] tensors.
            """
            iw, ih = mk("iw"), mk("ih")
            if ts_tgt:
                nc.vector._custom_dve(IXREL, out=iw[:], in0=prbx, in1=pltx,
                                      s0=trbx, s1=tltx)
                nc.vector._custom_dve(IXREL, out=ih[:], in0=prby, in1=plty,
                                      s0=trby, s1=tlty)
            else:
                a, b = mk("mma"), mk("mmb")
                nc.vector.tensor_tensor(out=a[:], in0=prbx, in1=trbx, op=TS.min)
                nc.vector.tensor_tensor(out=b[:], in0=pltx, in1=tltx, op=TS.max)
                nc.vector.tensor_tensor(out=a[:], in0=a[:], in1=b[:], op=TS.subtract)
                nc.vector.tensor_scalar(out=iw[:], in0=a[:], scalar1=0.0,
                                        scalar2=None, op0=TS.max)
                nc.vector.tensor_tensor(out=a[:], in0=prby, in1=trby, op=TS.min)
                nc.vector.tensor_tensor(out=b[:], in0=plty, in1=tlty, op=TS.max)
                nc.vector.tensor_tensor(out=a[:], in0=a[:], in1=b[:], op=TS.subtract)
                nc.vector.tensor_scalar(out=ih[:], in0=a[:], scalar1=0.0,
                                        scalar2=None, op0=TS.max)
            inter = mk("inter")
            nc.vector.tensor_tensor(out=inter[:], in0=iw[:], in1=ih[:], op=TS.mult)
            u = mk("u")
            if ts_tgt:
                nc.vector.tensor_scalar(out=u[:], in0=pa, scalar1=ta,
                                        scalar2=None, op0=TS.add)
            else:
                nc.vector.tensor_tensor(out=u[:], in0=pa, in1=ta, op=TS.add)
            nc.vector.tensor_tensor(out=u[:], in0=u[:], in1=inter[:], op=TS.subtract)
            nc.vector.tensor_scalar(out=u[:], in0=u[:], scalar1=float(EPS),
                                    scalar2=None, op0=TS.add)
            r0, r = mk("r0"), mk("r")
            nc.vector.reciprocal_approx_fast(out=r0[:], in_=u[:])
            nc.vector._custom_dve(dve_ops.RECIPROCAL_APPROX_NR, out=r[:], in0=u[:],
                                  in1=r0[:], s0=2.0)
            iou2 = mk("iou2")
            nc.vector.tensor_tensor(out=iou2[:], in0=inter[:], in1=r[:], op=TS.mult)
            # center distance
            dx, dy, cd = mk("dx"), mk("dy"), mk("cd")
            if ts_tgt:
                nc.vector.tensor_scalar(out=dx[:], in0=pcx, scalar1=tcx,
                                        scalar2=None, op0=TS.subtract)
                nc.vector.tensor_scalar(out=dy[:], in0=pcy, scalar1=tcy,
                                        scalar2=None, op0=TS.subtract)
            else:
                nc.vector.tensor_tensor(out=dx[:], in0=pcx, in1=tcx, op=TS.subtract)
                nc.vector.tensor_tensor(out=dy[:], in0=pcy, in1=tcy, op=TS.subtract)
            nc.vector.tensor_tensor(out=dx[:], in0=dx[:], in1=dx[:], op=TS.mult)
            nc.vector.tensor_tensor(out=dy[:], in0=dy[:], in1=dy[:], op=TS.mult)
            nc.vector.tensor_tensor(out=cd[:], in0=dx[:], in1=dy[:], op=TS.add)
            # enclosing box diagonal
            ex, ey, e2 = mk("ex"), mk("ey"), mk("e2")
            if ts_tgt:
                nc.vector.tensor_scalar(out=ex[:], in0=prbx, scalar1=trbx,
                                        scalar2=None, op0=TS.max)
                nc.vector.tensor_scalar(out=e2[:], in0=pltx, scalar1=tltx,
                                        scalar2=None, op0=TS.min)
                nc.vector.tensor_tensor(out=ex[:], in0=ex[:], in1=e2[:], op=TS.subtract)
                nc.vector.tensor_scalar(out=ey[:], in0=prby, scalar1=trby,
                                        scalar2=None, op0=TS.max)
                nc.vector.tensor_scalar(out=e2[:], in0=plty, scalar1=tlty,
                                        scalar2=None, op0=TS.min)
                nc.vector.tensor_tensor(out=ey[:], in0=ey[:], in1=e2[:], op=TS.subtract)
            else:
                nc.vector.tensor_tensor(out=ex[:], in0=prbx, in1=trbx, op=TS.max)
                nc.vector.tensor_tensor(out=e2[:], in0=pltx, in1=tltx, op=TS.min)
                nc.vector.tensor_tensor(out=ex[:], in0=ex[:], in1=e2[:], op=TS.subtract)
                nc.vector.tensor_tensor(out=ey[:], in0=prby, in1=trby, op=TS.max)
                nc.vector.tensor_tensor(out=e2[:], in0=plty, in1=tlty, op=TS.min)
                nc.vector.tensor_tensor(out=ey[:], in0=ey[:], in1=e2[:], op=TS.subtract)
            diag = mk("diag")
            nc.vector.tensor_tensor(out=ex[:], in0=ex[:], in1=ex[:], op=TS.mult)
            nc.vector.tensor_tensor(out=ey[:], in0=ey[:], in1=ey[:], op=TS.mult)
            nc.vector.tensor_tensor(out=diag[:], in0=ex[:], in1=ey[:], op=TS.add)
            nc.vector.tensor_scalar(out=diag[:], in0=diag[:], scalar1=float(EPS),
                                    scalar2=None, op0=TS.add)
            d0, dr = mk("d0"), mk("dr")
            nc.vector.reciprocal_approx_fast(out=d0[:], in_=diag[:])
            nc.vector._custom_dve(dve_ops.RECIPROCAL_APPROX_NR, out=dr[:],
                                  in0=diag[:], in1=d0[:], s0=2.0)
            cdd, diou = mk("cdd"), mk("diou")
            nc.vector.tensor_tensor(out=cdd[:], in0=cd[:], in1=dr[:], op=TS.mult)
            nc.vector.tensor_tensor(out=diou[:], in0=iou2[:], in1=cdd[:], op=TS.subtract)
            return diou

        paA, taA = t3("paA"), t3("taA")
        nc.vector.tensor_tensor(out=paA[:], in0=pw, in1=ph, op=TS.mult)
        nc.vector.tensor_tensor(out=taA[:], in0=tw, in1=th, op=TS.mult)
        mkA = lambda tag: sml.tile([128, NT], F32, tag="A_" + tag, name="A_" + tag)
        diouA = diou_common(pltx[:], plty[:], prbx[:], prby[:],
                            tltx[:], tlty[:], trbx[:], trby[:],
                            pcx, pcy, tcx, tcy, paA[:], taA[:], NT, mkA, False)
        nc.vector.tensor_tensor(out=outt[:, 0:NT], in0=diouA[:], in1=vld, op=TS.mult)

        # ---- DIoU tail B: all original rows (masked to degenerate) vs tgt0 ----
        fcx = predf[:, 0:64:8]
        fcy = predf[:, 1:64:8]
        fw = predf[:, 2:64:8]
        fh = predf[:, 3:64:8]
        fmask = predf[:, 4:64:8]
        mkB = lambda tag: sml.tile([128, 8], F32, tag="B_" + tag, name="B_" + tag)
        fhw, fhh = mkB("fhw"), mkB("fhh")
        nc.vector.tensor_scalar(out=fhw[:], in0=fw, scalar1=0.5, scalar2=None, op0=TS.mult)
        nc.vector.tensor_scalar(out=fhh[:], in0=fh, scalar1=0.5, scalar2=None, op0=TS.mult)
        fltx, flty, frbx, frby = mkB("fltx"), mkB("flty"), mkB("frbx"), mkB("frby")
        nc.vector.tensor_tensor(out=fltx[:], in0=fcx, in1=fhw[:], op=TS.subtract)
        nc.vector.tensor_tensor(out=frbx[:], in0=fcx, in1=fhw[:], op=TS.add)
        nc.vector.tensor_tensor(out=flty[:], in0=fcy, in1=fhh[:], op=TS.subtract)
        nc.vector.tensor_tensor(out=frby[:], in0=fcy, in1=fhh[:], op=TS.add)
        faB = mkB("faB")
        nc.vector.tensor_tensor(out=faB[:], in0=fw, in1=fh, op=TS.mult)
        diouB = diou_common(
            fltx[:], flty[:], frbx[:], frby[:],
            tgt0[:, 4:5], tgt0[:, 5:6], tgt0[:, 6:7], tgt0[:, 7:8],
            fcx, fcy, tgt0[:, 0:1], tgt0[:, 1:2],
            faB[:], tgt0[:, 8:9], 8, mkB, True)
        nc.vector.tensor_tensor(out=outt[:, NT:NT + 8], in0=diouB[:], in1=fmask,
                                op=TS.mult)

        nc.sync.dma_start(out=out_d[:], in_=outt[:])

    nc.compile()
    _BUILD_CACHE[key] = nc
    return nc


def _numpy_fallback(pred, tgt):
    """Exact f32 reimplementation of the reference (for inputs the compiled
    capacities can't hold)."""
    P, T = pred.shape[0], tgt.shape[0]
    if P != T:
        lt = np.maximum(pred[:, None, :2], tgt[None, :, :2])
        rb = np.minimum(pred[:, None, 2:], tgt[None, :, 2:])
        wh = np.clip(rb - lt, 0.0, None).astype(np.float32)
        inter = wh[..., 0] * wh[..., 1]
        pa = (pred[:, 2] - pred[:, 0]) * (pred[:, 3] - pred[:, 1])
        ta = (tgt[:, 2] - tgt[:, 0]) * (tgt[:, 3] - tgt[:, 1])
        union = pa[:, None] + ta[None, :] - inter
        iou = inter / (union + EPS)
        idx = np.argmax(iou, axis=1)
        tgt = tgt[idx]
    pc, ps = pred[:, :2], pred[:, 2:]
    tc, ts = tgt[:, :2], tgt[:, 2:]
    plt_, prb = pc - ps / 2, pc + ps / 2
    tlt, trb = tc - ts / 2, tc + ts / 2
    iwh = np.clip(np.minimum(prb, trb) - np.maximum(plt_, tlt), 0.0, None)
    inter = iwh[:, 0] * iwh[:, 1]
    pa = ps[:, 0] * ps[:, 1]
    ta = ts[:, 0] * ts[:, 1]
    iou = inter / (pa + ta - inter + EPS)
    cd = np.sum((pc - tc) ** 2, axis=1)
    ewh = np.maximum(prb, trb) - np.minimum(plt_, tlt)
    diag = np.sum(ewh ** 2, axis=1)
    diou = iou - cd / (diag + EPS)
    return np.float32(1.0) - np.float32(diou.mean(dtype=np.float64))


def kernel(pred_boxes, target_boxes):
    pred = np.ascontiguousarray(np.asarray(pred_boxes, dtype=np.float32))
    tgt = np.ascontiguousarray(np.asarray(target_boxes, dtype=np.float32))
    P, T = pred.shape[0], tgt.shape[0]

    # host-side compaction (degenerate boxes intersect nothing; see module doc)
    pw = pred[:, 2] - pred[:, 0]
    ph = pred[:, 3] - pred[:, 1]
    pa = pw * ph
    tw = tgt[:, 2] - tgt[:, 0]
    th = tgt[:, 3] - tgt[:, 1]
    ta = tw * th
    nd_p = (pw > 0) & (ph > 0)
    nd_t = (tw > 0) & (th > 0)
    pidx = np.nonzero(nd_p)[0]
    tidx = np.nonzero(nd_t)[0]
    Np, Nt = len(pidx), len(tidx)
    if P != 8192 or T < 1 or Np > N_CORES * P_CAP_CORE or Nt > T_CAP:
        return _numpy_fallback(pred, tgt)

    nc = _build_program()

    # compacted target arrays, replicated across partitions
    ct = tgt[tidx]  # [Nt, 4]
    ctp = np.zeros((T_CAP, 4), dtype=np.float32)
    ctp[:Nt] = ct
    tap = np.zeros((T_CAP,), dtype=np.float32)
    tap[:Nt] = ta[tidx]
    rep = lambda v: np.ascontiguousarray(np.broadcast_to(v[None, :], (128, T_CAP)))
    tx1r = rep(ctp[:, 0])
    ty1r = rep(ctp[:, 1])
    tx2r = rep(ctp[:, 2])
    ty2r = rep(ctp[:, 3])
    tar = rep(tap)
    ctab = np.zeros((T_CAP + 1, 4), dtype=np.float32)
    ctab[:T_CAP] = ctp
    ctab[T_CAP] = tgt[0]

    # tgt0 per-partition scalars (cxcywh interp for the DIoU part)
    t0 = tgt[0]
    half = (t0[2:4] * np.float32(0.5)).astype(np.float32)
    tgt0row = np.zeros((12,), dtype=np.float32)
    tgt0row[0:4] = t0
    tgt0row[4:6] = t0[0:2] - half
    tgt0row[6:8] = t0[0:2] + half
    tgt0row[8] = t0[2] * t0[3]
    tgt0 = np.ascontiguousarray(np.broadcast_to(tgt0row[None, :], (128, 12)))

    # compacted preds: shard evenly (contiguous) across cores, pad to capacity
    per_core = -(-Np // N_CORES)  # ceil
    in_maps = []
    rows_per_core = P // N_CORES
    for c in range(N_CORES):
        sl = pidx[c * per_core : (c + 1) * per_core]
        n = len(sl)
        predc = np.zeros((128, 8 * P_TILES), dtype=np.float32)
        for i in range(P_TILES):
            seg = sl[i * 128 : (i + 1) * 128]
            k = len(seg)
            if k:
                blk = np.zeros((128, 8), dtype=np.float32)
                blk[:k, 0:4] = pred[seg]
                blk[:k, 4] = pa[seg] + EPS
                blk[:k, 5] = 1.0
                predc[:, 8 * i : 8 * i + 8] = blk
        predf = np.zeros((128, 64), dtype=np.float32)
        base = c * rows_per_core
        for j in range(rows_per_core // 128):
            seg = slice(base + j * 128, base + (j + 1) * 128)
            predf[:, 8 * j : 8 * j + 4] = pred[seg]
            predf[:, 8 * j + 4] = (~nd_p[seg]).astype(np.float32)
        in_maps.append({
            "tx1r": tx1r, "ty1r": ty1r, "tx2r": tx2r, "ty2r": ty2r, "tar": tar,
            "predc": predc, "predf": predf, "ctab": ctab, "tgt0": tgt0,
        })

    trace = os.environ.get("BASS_DIOU_TRACE") == "1"
    res = run_bass_kernel_spmd(nc, in_maps, list(range(N_CORES)), trace=trace)
    global LAST_RESULTS
    LAST_RESULTS = res
    total = np.float64(0.0)
    for c in range(N_CORES):
        total += np.float64(res.results[c]["acc"].sum(dtype=np.float64))
    return np.float32(np.float32(1.0) - np.float32(total / P))


# revision 11
# speedup vs baseline: 5.5526x; 5.5526x over previous
"""DIoU loss (nms_detection) Trainium2 kernel.

Reference semantics: pairwise IoU [P,T] between pred_boxes (xyxy interp) and
target_boxes, argmax over targets per pred row (first-occurrence tie-break),
gather matched targets, DIoU (cxcywh interp) per row, loss = 1 - mean(diou).

Key algorithmic facts exploited (validated against the jax reference):
 1. A box with w<=0 or h<=0 (75% of uniform-random "boxes") has zero
    intersection with everything -> its whole IoU row/column is +-0.  A row
    whose max IoU is <= 0 argmaxes to index 0 (first occurrence among ties).
    So only non-degenerate preds x non-degenerate targets need the dense
    pairwise block (~1/16 of the matrix), and every row with max<=0 matches
    target_boxes[0].
 2. iou = inter/(pa+ta-inter+eps) is a monotone function of
    iou' = inter/(pa+ta+eps) (g = f/(1+f)), so argmax(iou) == argmax(iou').
    This removes one full-width pass (the union) from the hot loop.

Sharding: compacted non-degenerate pred rows are split evenly across the 8
cores (data-parallel rows of the pairwise block, per the sharding hint); each
core holds the full compacted target set, computes its block + local DIoU
partial sums; the final (tiny) reduction over 8x[128,16] partials is done on
host, equivalent to the suggested all-reduce of the mean.
"""

import os
import numpy as np
from contextlib import ExitStack

import concourse.bass as bass
import concourse.bacc as bacc
import concourse.mybir as mybir
from concourse import tile
from concourse.bass_utils import run_bass_kernel_spmd

N_CORES = 8
T_CAP = 1280      # compacted-target capacity (multiple of 128)
P_TILES = 3       # compacted-pred tiles per core -> 384 rows/core, 3072 total
P_CAP_CORE = 128 * P_TILES
EPS = np.float32(1e-7)
F32 = mybir.dt.float32
U32 = mybir.dt.uint32

# ---------------------------------------------------------------------------
# Custom DVE ops (registered at import; names unique, appended after stock OPS)
# ---------------------------------------------------------------------------
from concourse.dve_spec import Spec, Src0, Src1, C0, C1, lower, relu, minn, maxx, AluOp
import concourse.dve_ops as dve_ops
from concourse.dve_ops import DveOp, OPS
from concourse.dve_uop import DveOpSpec


def _register_dve(name, spec):
    for op in OPS:
        if op.name == name:
            return op
    shas = {}
    for ver in ("v3", "v4"):
        uops = lower(spec, ver=ver)
        shas[ver] = DveOpSpec(name=name, opcode=0, uops=uops, rd1_en=True).sha(ver)
    op = DveOp(name, spec, subdim=False, uops_sha=shas)
    OPS.append(op)
    dve_ops.CUSTOM_DVE_SPECS[name] = spec
    dve_ops._SUB_OPCODE_FOR_NAME[name] = dve_ops._CUSTOM_DVE_ROW_BASE + len(OPS) - 1
    assert dve_ops._SUB_OPCODE_FOR_NAME[name] < 0x20
    return op


# relu(min(Src0, s0) - max(Src1, s1)): clipped 1-D interval overlap in one pass
IXREL = _register_dve(
    "IXREL_ANT",
    Spec(
        body=relu(minn(Src0, C0) - maxx(Src1, C1)),
        reference=lambda in0, in1, s0, s1, imm2: np.maximum(
            np.minimum(in0, s0) - np.maximum(in1, s1), 0.0
        ),
    ),
)

# out = Src0*Src1, accum_out = max(out) (tensor_tensor_reduce crashes the
# device on this toolchain; this custom op is the working replacement)
MUL_RMAX = _register_dve(
    "MUL_RMAX_ANT",
    Spec(
        body=Src0 * Src1,
        accum=AluOp.MAX,
        reference=lambda in0, in1, s0, s1, imm2: in0 * in1,
    ),
)

_BUILD_CACHE = {}


def _build_program():
    key = (T_CAP, P_TILES)
    if key in _BUILD_CACHE:
        return _BUILD_CACHE[key]

    nc = bacc.Bacc("TRN2", target_bir_lowering=False, debug=False,
                   num_devices=N_CORES)

    din = lambda n, s: nc.dram_tensor(n, s, F32, kind="ExternalInput").ap()
    tx1r_d = din("tx1r", [128, T_CAP])
    ty1r_d = din("ty1r", [128, T_CAP])
    tx2r_d = din("tx2r", [128, T_CAP])
    ty2r_d = din("ty2r", [128, T_CAP])
    tar_d = din("tar", [128, T_CAP])
    predc_d = din("predc", [128, 8 * P_TILES])   # [x1,y1,x2,y2,pa_eps,valid,0,0]*3
    predf_d = din("predf", [128, 64])            # 8 tiles x [cx,cy,w,h,notnd,0,0,0]
    ctab_d = din("ctab", [T_CAP + 1, 4])         # compacted targets + tgt0 at row T_CAP
    tgt0_d = din("tgt0", [128, 12])  # [cx,cy,w,h,ltx,lty,rbx,rby,ta0,0,0,0]
    out_d = nc.dram_tensor("acc", [128, P_TILES + 8], F32, kind="ExternalOutput").ap()

    with tile.TileContext(nc) as tc, ExitStack() as ctx:
        rep = ctx.enter_context(tc.tile_pool(name="rep", bufs=1))
        big = ctx.enter_context(tc.tile_pool(name="big", bufs=2))
        sml = ctx.enter_context(tc.tile_pool(name="sml", bufs=1))

        tx1r = rep.tile([128, T_CAP], F32, tag="tx1r", name="tx1r")
        ty1r = rep.tile([128, T_CAP], F32, tag="ty1r", name="ty1r")
        tx2r = rep.tile([128, T_CAP], F32, tag="tx2r", name="tx2r")
        ty2r = rep.tile([128, T_CAP], F32, tag="ty2r", name="ty2r")
        tar = rep.tile([128, T_CAP], F32, tag="tar", name="tar")
        predc = sml.tile([128, 8 * P_TILES], F32, tag="predc", name="predc")
        predf = sml.tile([128, 64], F32, tag="predf", name="predf")
        tgt0 = sml.tile([128, 12], F32, tag="tgt0", name="tgt0")
        nc.sync.dma_start(out=tx1r[:], in_=tx1r_d[:])
        nc.sync.dma_start(out=ty1r[:], in_=ty1r_d[:])
        nc.sync.dma_start(out=tx2r[:], in_=tx2r_d[:])
        nc.sync.dma_start(out=ty2r[:], in_=ty2r_d[:])
        nc.sync.dma_start(out=tar[:], in_=tar_d[:])
        nc.sync.dma_start(out=predc[:], in_=predc_d[:])
        nc.sync.dma_start(out=predf[:], in_=predf_d[:])
        nc.sync.dma_start(out=tgt0[:], in_=tgt0_d[:])

        selall = sml.tile([128, P_TILES], F32, tag="selall", name="selall")
        gtall = sml.tile([128, 4 * P_TILES], F32, tag="gtall", name="gtall")
        outt = sml.tile([128, P_TILES + 8], F32, tag="outt", name="outt")

        TS = mybir.AluOpType

        # ---- pairwise block: per 128-row pred tile ----
        for i in range(P_TILES):
            px1 = predc[:, 8 * i + 0 : 8 * i + 1]
            py1 = predc[:, 8 * i + 1 : 8 * i + 2]
            px2 = predc[:, 8 * i + 2 : 8 * i + 3]
            py2 = predc[:, 8 * i + 3 : 8 * i + 4]
            pae = predc[:, 8 * i + 4 : 8 * i + 5]

            S = big.tile([128, T_CAP], F32, tag="S", name="S")
            J0 = big.tile([128, T_CAP], F32, tag="J0", name="J0")
            J = big.tile([128, T_CAP], F32, tag="J", name="J")
            ix = big.tile([128, T_CAP], F32, tag="ix", name="ix")
            iy = big.tile([128, T_CAP], F32, tag="iy", name="iy")
            inter = big.tile([128, T_CAP], F32, tag="inter", name="inter")
            iou = big.tile([128, T_CAP], F32, tag="iou", name="iou")
            m = sml.tile([128, 1], F32, tag=f"m{i}", name=f"m{i}")
            m8 = sml.tile([128, 8], F32, tag=f"m8{i}", name=f"m8{i}")
            ti8 = sml.tile([128, 8], U32, tag=f"ti8{i}", name=f"ti8{i}")
            idxf = sml.tile([128, 1], F32, tag=f"idxf{i}", name=f"idxf{i}")
            idxs = sml.tile([128, 1], F32, tag=f"idxs{i}", name=f"idxs{i}")
            idxu = sml.tile([128, 1], U32, tag=f"idxu{i}", name=f"idxu{i}")

            # S = max(tar + (pa+eps), eps)  (>0 wherever it matters)
            nc.vector.tensor_scalar(out=S[:], in0=tar[:], scalar1=pae,
                                    scalar2=float(EPS), op0=TS.add, op1=TS.max)
            nc.vector.reciprocal_approx_fast(out=J0[:], in_=S[:])
            nc.vector._custom_dve(  # one Newton step -> ~2 ULP
                dve_ops.RECIPROCAL_APPROX_NR, out=J[:], in0=S[:], in1=J0[:], s0=2.0
            )
            # ix = relu(min(px2,tx2) - max(px1,tx1)); same for y
            nc.vector._custom_dve(IXREL, out=ix[:], in0=tx2r[:], in1=tx1r[:],
                                  s0=px2, s1=px1)
            nc.vector._custom_dve(IXREL, out=iy[:], in0=ty2r[:], in1=ty1r[:],
                                  s0=py2, s1=py1)
            nc.vector.tensor_tensor(out=inter[:], in0=ix[:], in1=iy[:], op=TS.mult)
            # iou' = inter * (1/S); m = row max
            nc.vector._custom_dve(MUL_RMAX, out=iou[:], in0=inter[:], in1=J[:],
                                  accum_out=m[:])
            # first-occurrence argmax of iou' (== ref argmax when m > 0)
            nc.vector.tensor_copy(out=m8[:], in_=m[:, 0:1].broadcast_to((128, 8)))
            nc.vector.max_index(out=ti8[:], in_max=m8[:], in_values=iou[:])
            # sel = (m > 0); idx = sel ? argmax : T_CAP (row T_CAP of ctab = tgt0)
            nc.vector.tensor_scalar(out=selall[:, i : i + 1], in0=m[:], scalar1=0.0,
                                    scalar2=None, op0=TS.is_gt)
            nc.vector.tensor_copy(out=idxf[:], in_=ti8[:, 0:1])
            nc.vector.tensor_scalar(out=idxs[:], in0=idxf[:], scalar1=float(T_CAP),
                                    scalar2=None, op0=TS.subtract)
            nc.vector.tensor_tensor(out=idxs[:], in0=idxs[:],
                                    in1=selall[:, i : i + 1], op=TS.mult)
            nc.vector.tensor_scalar(out=idxs[:], in0=idxs[:], scalar1=float(T_CAP),
                                    scalar2=None, op0=TS.add)
            nc.vector.tensor_copy(out=idxu[:], in_=idxs[:])
            nc.gpsimd.indirect_dma_start(
                out=gtall[:, 4 * i : 4 * i + 4], out_offset=None, in_=ctab_d[:],
                in_offset=bass.IndirectOffsetOnAxis(ap=idxu[:, 0:1], axis=0),
            )

        # ---- DIoU tail A: compacted rows vs gathered targets, [128, P_TILES] ----
        NT = P_TILES
        pcx = predc[:, 0 : 8 * NT : 8]
        pcy = predc[:, 1 : 8 * NT : 8]
        pw = predc[:, 2 : 8 * NT : 8]
        ph = predc[:, 3 : 8 * NT : 8]
        vld = predc[:, 5 : 8 * NT : 8]
        tcx = gtall[:, 0 : 4 * NT : 4]
        tcy = gtall[:, 1 : 4 * NT : 4]
        tw = gtall[:, 2 : 4 * NT : 4]
        th = gtall[:, 3 : 4 * NT : 4]

        def t3(tag):
            return sml.tile([128, NT], F32, tag=tag, name=tag)

        phw, phh, thw, thh = t3("phw"), t3("phh"), t3("thw"), t3("thh")
        nc.vector.tensor_scalar(out=phw[:], in0=pw, scalar1=0.5, scalar2=None, op0=TS.mult)
        nc.vector.tensor_scalar(out=phh[:], in0=ph, scalar1=0.5, scalar2=None, op0=TS.mult)
        nc.vector.tensor_scalar(out=thw[:], in0=tw, scalar1=0.5, scalar2=None, op0=TS.mult)
        nc.vector.tensor_scalar(out=thh[:], in0=th, scalar1=0.5, scalar2=None, op0=TS.mult)
        pltx, plty, prbx, prby = t3("pltx"), t3("plty"), t3("prbx"), t3("prby")
        tltx, tlty, trbx, trby = t3("tltx"), t3("tlty"), t3("trbx"), t3("trby")
        nc.vector.tensor_tensor(out=pltx[:], in0=pcx, in1=phw[:], op=TS.subtract)
        nc.vector.tensor_tensor(out=prbx[:], in0=pcx, in1=phw[:], op=TS.add)
        nc.vector.tensor_tensor(out=plty[:], in0=pcy, in1=phh[:], op=TS.subtract)
        nc.vector.tensor_tensor(out=prby[:], in0=pcy, in1=phh[:], op=TS.add)
        nc.vector.tensor_tensor(out=tltx[:], in0=tcx, in1=thw[:], op=TS.subtract)
        nc.vector.tensor_tensor(out=trbx[:], in0=tcx, in1=thw[:], op=TS.add)
        nc.vector.tensor_tensor(out=tlty[:], in0=tcy, in1=thh[:], op=TS.subtract)
        nc.vector.tensor_tensor(out=trby[:], in0=tcy, in1=thh[:], op=TS.add)

        def diou_common(pltx, plty, prbx, prby, tltx, tlty, trbx, trby,
                        pcx, pcy, tcx, tcy, pa, ta, n, mk, ts_tgt):
            """Emits diou into a [128, n] tile and returns it.

            All args are ready-to-use APs.  ts_tgt: True when the t* box
            sides / centers / area are per-partition [128,1] scalars
            (tensor_scalar path), False when they are [128, n] tensors.
            """
            iw, ih = mk("iw"), mk("ih")
            if ts_tgt:
                nc.vector._custom_dve(IXREL, out=iw[:], in0=prbx, in1=pltx,
                                      s0=trbx, s1=tltx)
                nc.vector._custom_dve(IXREL, out=ih[:], in0=prby, in1=plty,
                                      s0=trby, s1=tlty)
            else:
                a, b = mk("mma"), mk("mmb")
                nc.vector.tensor_tensor(out=a[:], in0=prbx, in1=trbx, op=TS.min)
                nc.vector.tensor_tensor(out=b[:], in0=pltx, in1=tltx, op=TS.max)
                nc.vector.tensor_tensor(out=a[:], in0=a[:], in1=b[:], op=TS.subtract)
                nc.vector.tensor_scalar(out=iw[:], in0=a[:], scalar1=0.0,
                                        scalar2=None, op0=TS.max)
                nc.vector.tensor_tensor(out=a[:], in0=prby, in1=trby, op=TS.min)
                nc.vector.tensor_tensor(out=b[:], in0=plty, in1=tlty, op=TS.max)
                nc.vector.tensor_tensor(out=a[:], in0=a[:], in1=b[:], op=TS.subtract)
                nc.vector.tensor_scalar(out=ih[:], in0=a[:], scalar1=0.0,
                                        scalar2=None, op0=TS.max)
            inter = mk("inter")
            nc.vector.tensor_tensor(out=inter[:], in0=iw[:], in1=ih[:], op=TS.mult)
            u = mk("u")
            if ts_tgt:
                nc.vector.tensor_scalar(out=u[:], in0=pa, scalar1=ta,
                                        scalar2=None, op0=TS.add)
            else:
                nc.vector.tensor_tensor(out=u[:], in0=pa, in1=ta, op=TS.add)
            nc.vector.tensor_tensor(out=u[:], in0=u[:], in1=inter[:], op=TS.subtract)
            nc.vector.tensor_scalar(out=u[:], in0=u[:], scalar1=float(EPS),
                                    scalar2=None, op0=TS.add)
            r0, r = mk("r0"), mk("r")
            nc.vector.reciprocal_approx_fast(out=r0[:], in_=u[:])
            nc.vector._custom_dve(dve_ops.RECIPROCAL_APPROX_NR, out=r[:], in0=u[:],
                                  in1=r0[:], s0=2.0)
            iou2 = mk("iou2")
            nc.vector.tensor_tensor(out=iou2[:], in0=inter[:], in1=r[:], op=TS.mult)
            # center distance
            dx, dy, cd = mk("dx"), mk("dy"), mk("cd")
            if ts_tgt:
                nc.vector.tensor_scalar(out=dx[:], in0=pcx, scalar1=tcx,
                                        scalar2=None, op0=TS.subtract)
                nc.vector.tensor_scalar(out=dy[:], in0=pcy, scalar1=tcy,
                                        scalar2=None, op0=TS.subtract)
            else:
                nc.vector.tensor_tensor(out=dx[:], in0=pcx, in1=tcx, op=TS.subtract)
                nc.vector.tensor_tensor(out=dy[:], in0=pcy, in1=tcy, op=TS.subtract)
            nc.vector.tensor_tensor(out=dx[:], in0=dx[:], in1=dx[:], op=TS.mult)
            nc.vector.tensor_tensor(out=dy[:], in0=dy[:], in1=dy[:], op=TS.mult)
            nc.vector.tensor_tensor(out=cd[:], in0=dx[:], in1=dy[:], op=TS.add)
            # enclosing box diagonal
            ex, ey, e2 = mk("ex"), mk("ey"), mk("e2")
            if ts_tgt:
                nc.vector.tensor_scalar(out=ex[:], in0=prbx, scalar1=trbx,
                                        scalar2=None, op0=TS.max)
                nc.vector.tensor_scalar(out=e2[:], in0=pltx, scalar1=tltx,
                                        scalar2=None, op0=TS.min)
                nc.vector.tensor_tensor(out=ex[:], in0=ex[:], in1=e2[:], op=TS.subtract)
                nc.vector.tensor_scalar(out=ey[:], in0=prby, scalar1=trby,
                                        scalar2=None, op0=TS.max)
                nc.vector.tensor_scalar(out=e2[:], in0=plty, scalar1=tlty,
                                        scalar2=None, op0=TS.min)
                nc.vector.tensor_tensor(out=ey[:], in0=ey[:], in1=e2[:], op=TS.subtract)
            else:
                nc.vector.tensor_tensor(out=ex[:], in0=prbx, in1=trbx, op=TS.max)
                nc.vector.tensor_tensor(out=e2[:], in0=pltx, in1=tltx, op=TS.min)
                nc.vector.tensor_tensor(out=ex[:], in0=ex[:], in1=e2[:], op=TS.subtract)
                nc.vector.tensor_tensor(out=ey[:], in0=prby, in1=trby, op=TS.max)
                nc.vector.tensor_tensor(out=e2[:], in0=plty, in1=tlty, op=TS.min)
                nc.vector.tensor_tensor(out=ey[:], in0=ey[:], in1=e2[:], op=TS.subtract)
            diag = mk("diag")
            nc.vector.tensor_tensor(out=ex[:], in0=ex[:], in1=ex[:], op=TS.mult)
            nc.vector.tensor_tensor(out=ey[:], in0=ey[:], in1=ey[:], op=TS.mult)
            nc.vector.tensor_tensor(out=diag[:], in0=ex[:], in1=ey[:], op=TS.add)
            nc.vector.tensor_scalar(out=diag[:], in0=diag[:], scalar1=float(EPS),
                                    scalar2=None, op0=TS.add)
            d0, dr = mk("d0"), mk("dr")
            nc.vector.reciprocal_approx_fast(out=d0[:], in_=diag[:])
            nc.vector._custom_dve(dve_ops.RECIPROCAL_APPROX_NR, out=dr[:],
                                  in0=diag[:], in1=d0[:], s0=2.0)
            cdd, diou = mk("cdd"), mk("diou")
            nc.vector.tensor_tensor(out=cdd[:], in0=cd[:], in1=dr[:], op=TS.mult)
            nc.vector.tensor_tensor(out=diou[:], in0=iou2[:], in1=cdd[:], op=TS.subtract)
            return diou

        paA, taA = t3("paA"), t3("taA")
        nc.vector.tensor_tensor(out=paA[:], in0=pw, in1=ph, op=TS.mult)
        nc.vector.tensor_tensor(out=taA[:], in0=tw, in1=th, op=TS.mult)
        mkA = lambda tag: sml.tile([128, NT], F32, tag="A_" + tag, name="A_" + tag)
        diouA = diou_common(pltx[:], plty[:], prbx[:], prby[:],
                            tltx[:], tlty[:], trbx[:], trby[:],
                            pcx, pcy, tcx, tcy, paA[:], taA[:], NT, mkA, False)
        nc.vector.tensor_tensor(out=outt[:, 0:NT], in0=diouA[:], in1=vld, op=TS.mult)

        # ---- DIoU tail B: all original rows (masked to degenerate) vs tgt0 ----
        fcx = predf[:, 0:64:8]
        fcy = predf[:, 1:64:8]
        fw = predf[:, 2:64:8]
        fh = predf[:, 3:64:8]
        fmask = predf[:, 4:64:8]
        mkB = lambda tag: sml.tile([128, 8], F32, tag="B_" + tag, name="B_" + tag)
        fhw, fhh = mkB("fhw"), mkB("fhh")
        nc.vector.tensor_scalar(out=fhw[:], in0=fw, scalar1=0.5, scalar2=None, op0=TS.mult)
        nc.vector.tensor_scalar(out=fhh[:], in0=fh, scalar1=0.5, scalar2=None, op0=TS.mult)
        fltx, flty, frbx, frby = mkB("fltx"), mkB("flty"), mkB("frbx"), mkB("frby")
        nc.vector.tensor_tensor(out=fltx[:], in0=fcx, in1=fhw[:], op=TS.subtract)
        nc.vector.tensor_tensor(out=frbx[:], in0=fcx, in1=fhw[:], op=TS.add)
        nc.vector.tensor_tensor(out=flty[:], in0=fcy, in1=fhh[:], op=TS.subtract)
        nc.vector.tensor_tensor(out=frby[:], in0=fcy, in1=fhh[:], op=TS.add)
        faB = mkB("faB")
        nc.vector.tensor_tensor(out=faB[:], in0=fw, in1=fh, op=TS.mult)
        diouB = diou_common(
            fltx[:], flty[:], frbx[:], frby[:],
            tgt0[:, 4:5], tgt0[:, 5:6], tgt0[:, 6:7], tgt0[:, 7:8],
            fcx, fcy, tgt0[:, 0:1], tgt0[:, 1:2],
            faB[:], tgt0[:, 8:9], 8, mkB, True)
        nc.vector.tensor_tensor(out=outt[:, NT:NT + 8], in0=diouB[:], in1=fmask,
                                op=TS.mult)

        nc.sync.dma_start(out=out_d[:], in_=outt[:])

    nc.compile()
    _BUILD_CACHE[key] = nc
    return nc


def _numpy_fallback(pred, tgt):
    """Exact f32 reimplementation of the reference (for inputs the compiled
    capacities can't hold)."""
    P, T = pred.shape[0], tgt.shape[0]
    if P != T:
        lt = np.maximum(pred[:, None, :2], tgt[None, :, :2])
        rb = np.minimum(pred[:, None, 2:], tgt[None, :, 2:])
        wh = np.clip(rb - lt, 0.0, None).astype(np.float32)
        inter = wh[..., 0] * wh[..., 1]
        pa = (pred[:, 2] - pred[:, 0]) * (pred[:, 3] - pred[:, 1])
        ta = (tgt[:, 2] - tgt[:, 0]) * (tgt[:, 3] - tgt[:, 1])
        union = pa[:, None] + ta[None, :] - inter
        iou = inter / (union + EPS)
        idx = np.argmax(iou, axis=1)
        tgt = tgt[idx]
    pc, ps = pred[:, :2], pred[:, 2:]
    tc, ts = tgt[:, :2], tgt[:, 2:]
    plt_, prb = pc - ps / 2, pc + ps / 2
    tlt, trb = tc - ts / 2, tc + ts / 2
    iwh = np.clip(np.minimum(prb, trb) - np.maximum(plt_, tlt), 0.0, None)
    inter = iwh[:, 0] * iwh[:, 1]
    pa = ps[:, 0] * ps[:, 1]
    ta = ts[:, 0] * ts[:, 1]
    iou = inter / (pa + ta - inter + EPS)
    cd = np.sum((pc - tc) ** 2, axis=1)
    ewh = np.maximum(prb, trb) - np.minimum(plt_, tlt)
    diag = np.sum(ewh ** 2, axis=1)
    diou = iou - cd / (diag + EPS)
    return np.float32(1.0) - np.float32(diou.mean(dtype=np.float64))


def host_prep(pred, tgt):
    """Compaction + per-core input packing.  Returns the 8 in_maps, or None
    when the compiled capacities can't hold this input."""
    P, T = pred.shape[0], tgt.shape[0]

    # host-side compaction (degenerate boxes intersect nothing; see module doc)
    pw = pred[:, 2] - pred[:, 0]
    ph = pred[:, 3] - pred[:, 1]
    pa = pw * ph
    tw = tgt[:, 2] - tgt[:, 0]
    th = tgt[:, 3] - tgt[:, 1]
    ta = tw * th
    nd_p = (pw > 0) & (ph > 0)
    nd_t = (tw > 0) & (th > 0)
    pidx = np.nonzero(nd_p)[0]
    tidx = np.nonzero(nd_t)[0]
    Np, Nt = len(pidx), len(tidx)
    if P != 8192 or T < 1 or Np > N_CORES * P_CAP_CORE or Nt > T_CAP:
        return None

    # compacted target arrays, replicated across partitions
    ct = tgt[tidx]  # [Nt, 4]
    ctp = np.zeros((T_CAP, 4), dtype=np.float32)
    ctp[:Nt] = ct
    tap = np.zeros((T_CAP,), dtype=np.float32)
    tap[:Nt] = ta[tidx]
    rep = lambda v: np.ascontiguousarray(np.broadcast_to(v[None, :], (128, T_CAP)))
    tx1r = rep(ctp[:, 0])
    ty1r = rep(ctp[:, 1])
    tx2r = rep(ctp[:, 2])
    ty2r = rep(ctp[:, 3])
    tar = rep(tap)
    ctab = np.zeros((T_CAP + 1, 4), dtype=np.float32)
    ctab[:T_CAP] = ctp
    ctab[T_CAP] = tgt[0]

    # tgt0 per-partition scalars (cxcywh interp for the DIoU part)
    t0 = tgt[0]
    half = (t0[2:4] * np.float32(0.5)).astype(np.float32)
    tgt0row = np.zeros((12,), dtype=np.float32)
    tgt0row[0:4] = t0
    tgt0row[4:6] = t0[0:2] - half
    tgt0row[6:8] = t0[0:2] + half
    tgt0row[8] = t0[2] * t0[3]
    tgt0 = np.ascontiguousarray(np.broadcast_to(tgt0row[None, :], (128, 12)))

    # compacted preds: shard evenly (contiguous) across cores, pad to capacity
    per_core = -(-Np // N_CORES)  # ceil
    in_maps = []
    rows_per_core = P // N_CORES
    for c in range(N_CORES):
        sl = pidx[c * per_core : (c + 1) * per_core]
        n = len(sl)
        predc = np.zeros((128, 8 * P_TILES), dtype=np.float32)
        for i in range(P_TILES):
            seg = sl[i * 128 : (i + 1) * 128]
            k = len(seg)
            if k:
                blk = np.zeros((128, 8), dtype=np.float32)
                blk[:k, 0:4] = pred[seg]
                blk[:k, 4] = pa[seg] + EPS
                blk[:k, 5] = 1.0
                predc[:, 8 * i : 8 * i + 8] = blk
        predf = np.zeros((128, 64), dtype=np.float32)
        base = c * rows_per_core
        for j in range(rows_per_core // 128):
            seg = slice(base + j * 128, base + (j + 1) * 128)
            predf[:, 8 * j : 8 * j + 4] = pred[seg]
            predf[:, 8 * j + 4] = (~nd_p[seg]).astype(np.float32)
        in_maps.append({
            "tx1r": tx1r, "ty1r": ty1r, "tx2r": tx2r, "ty2r": ty2r, "tar": tar,
            "predc": predc, "predf": predf, "ctab": ctab, "tgt0": tgt0,
        })
    return in_maps


def kernel(pred_boxes, target_boxes):
    pred = np.ascontiguousarray(np.asarray(pred_boxes, dtype=np.float32))
    tgt = np.ascontiguousarray(np.asarray(target_boxes, dtype=np.float32))
    P = pred.shape[0]

    in_maps = host_prep(pred, tgt)
    if in_maps is None:
        return _numpy_fallback(pred, tgt)
    nc = _build_program()

    trace = os.environ.get("BASS_DIOU_TRACE") == "1"
    res = run_bass_kernel_spmd(nc, in_maps, list(range(N_CORES)), trace=trace)
    global LAST_RESULTS
    LAST_RESULTS = res
    total = np.float64(0.0)
    for c in range(N_CORES):
        total += np.float64(res.results[c]["acc"].sum(dtype=np.float64))
    return np.float32(np.float32(1.0) - np.float32(total / P))
